# revision 49
# baseline (speedup 1.0000x reference)
"""Multi-head self-attention Trainium2 kernel (Bass/Tile).

Problem: x:(8,256,32,32), 8 heads, head_dim=32, N=H*W=1024.
Sharding: data-parallel over batch B=8 -> one batch element per NeuronCore.

Per-core math (b fixed, X = x[b] as (C=256, N=1024)):
  q = (Wq@X + bq)/sqrt(hd) ; k = Wk@X ; v = Wv@X
  S[n,m] = qhat[:,n]k[:,m]      (bk-term constant along softmax axis -> drops;
                                 bq folded directly into qhat)
  P = softmax_m(S) ; O[d,n] = sum_m P[n,m] v[d,m]
  out = Wo@(O/den) + (Wo@bv + bo) + X[b]

Engine plan (cost-model: engines serialize; PE time = sum of moving-free
rows x pe_cycle; ACT time = free_size x 0.83ns):
  - PE: Q/K/V projections, S = K^T Q per (head, m-chunk), AV with
    ones-augmented V (denominator for free), output projection.
  - exp split 3 ways: ACT exact Exp; DVE + GpSimd use a Schraudolph
    fast-exp (int16 bit trick writing bf16 bit patterns) -- final rel err
    ~1e-3, way below the 2e-2 gate.
  - Pair k+1's S/exp/AV overlaps pair k's normalization; psO is copied to
    SBUF at pair end to free PSUM banks for a 3-deep S->exp pipeline.
"""

import math
import os

import numpy as np

import concourse.bass as bass
import concourse.mybir as mybir
import concourse.tile as tile
from concourse import bacc
from concourse.bass_utils import run_bass_kernel_spmd

F32 = mybir.dt.float32
F32R = mybir.dt.float32r
BF16 = mybir.dt.bfloat16
I16 = mybir.dt.int16
I8 = mybir.dt.int8
F8E4 = mybir.dt.float8e4
F8E5 = mybir.dt.float8e5
DROW = mybir.MatmulPerfMode.DoubleRow
EXP = mybir.ActivationFunctionType.Exp
IDENT = mybir.ActivationFunctionType.Identity
MULT = mybir.AluOpType.mult
ADD = mybir.AluOpType.add

NH = 8          # heads
HD = 32         # head dim
C = 256         # channels
N = 1024        # H*W
NCORES = 8

# Schraudolph fast-exp constants (bf16 bit pattern via int16):
#   i16 = trunc(x * 2^7/ln2 + (127*2^7 - C_ADJ)); bf16 = bitcast(i16)
SCH_A = float(2.0**7 / math.log(2.0))
SCH_B = float(127.0 * 2.0**7 - 0.5 * 2.0**4)
# e5m2 variant (int8 bit pattern): i8 = trunc(S*4/ln2 + (15*4 - 0.4))
SCH8_A = float(4.0 / math.log(2.0))
SCH8_B = float(15.0 * 4.0 - 0.4)

_NC = None          # cached compiled Bass module
LAST_RESULTS = None  # BassKernelResults of most recent run (for test.py)


def _exp_engine_schedule():
    """Greedy-balanced assignment of the 64 exp tiles to ACT/DVE/Pool,
    tracking each engine's other standing work as virtual load so no engine
    develops a backlog. Pair-end tiles (mc==7) are pinned to ACT: their
    latency gates the next pair's PSUM slots and ACT is fastest."""
    # GPSIMD cannot touch PSUM, so exp (PSUM source) splits across ACT and
    # DVE only; Pool gets the SBUF-side normalize work instead.
    cost = {"A": 920.0, "D": 1230.0}
    load = {"A": 2100.0 + 3400.0, "D": 2400.0}
    out = []
    for i in range(64):
        eng = min(load, key=lambda e: load[e] + cost[e])
        load[eng] += cost[eng]
        out.append(eng)
        if i % 16 == 15:  # per-pair: ACT pr/OST copies, DVE recips
            load["A"] += 2600.0
            load["D"] += 1400.0
        if i == 16 + 2:  # t=1 q/k projection copies during pair 1
            load["A"] += 2100.0
            load["D"] += 2400.0
    return out


def _exp_emit(nc, eng, et, ps):
    if eng == "A":
        nc.scalar.activation(et[:], ps[:], EXP)
    else:
        nc.vector.tensor_scalar(et[:].bitcast(I16), ps[:], SCH_A, SCH_B, MULT, ADD)


def _emit(tc, io):
    nc = tc.nc
    import contextlib

    ctx = contextlib.ExitStack()
    with ctx:
        pers = ctx.enter_context(tc.tile_pool(name="pers", bufs=1))
        etp = ctx.enter_context(tc.tile_pool(name="etp", bufs=10))
        psp = ctx.enter_context(tc.tile_pool(name="psp", bufs=2, space="PSUM"))

        def ptile(name, shape, dtype=F32):
            return pers.tile(shape, dtype, tag=name, name=name)

        def bigps(name, shape=None):
            return psp.tile(shape or [128, N], F32, tag="big", bufs=3, name=name)

        # warm the ACT exp table during the DMA window
        warm = ptile("warm", [1, 32])
        nc.vector.memset(warm[:], 0.0)
        nc.scalar.activation(warm[:], warm[:], EXP)

        # ---------------- load inputs (X first: it gates everything) -------
        # X as 4 half-tiles so projections can start after ~0.5MB.
        # bf16 staging copies of X and the packed weights gate the
        # projections (half the DMA bytes of f32); the f32 X only feeds the
        # residual add at the very end, so it streams in behind them.
        XB = [ptile(f"XB{i}", [128, 1024], BF16) for i in range(2)]
        X = [ptile(f"X{q}", [128, 512], F32R) for q in range(4)]
        # all four weights in one bf16 [128, 1024] tile per channel half:
        # cols 0:256 Wq^T/s | 256:512 Wk^T | 512:768 Wv^T | 768:1024 Wo^T
        W4 = [ptile(f"W4_{i}", [128, 1024], BF16) for i in range(2)]
        WQT = [W4[i][:, 0:256] for i in range(2)]
        WKT = [W4[i][:, 256:512] for i in range(2)]
        WVT = [W4[i][:, 512:768] for i in range(2)]
        WOT = [W4[i][:, 768:1024] for i in range(2)]
        MISC = ptile("MISC", [128, 4])
        BQ = MISC[:, 0:2]    # bq/sqrt(hd) columns per ch-tile
        BO2 = MISC[:, 2:4]   # (Wo@bv + bo) columns per ch-tile
        # broadcast map: row 0 -> pr rows 0-32, row 1 -> pr rows 64-96
        OH97 = ptile("OH97", [34, 97], F32R)  # map duplicated at rows 0/32

        def xdma(q):
            kc, jn = q // 2, q % 2
            nc.sync.dma_start(
                X[q][:], io["xb"][kc * 128 : (kc + 1) * 128, jn * 512 : (jn + 1) * 512]
            )

        # order by first use; HWDGE dispatch is ~625ns SERIAL per dma_start,
        # so few big transfers beat many small ones.
        nc.sync.dma_start(XB[0][:], io["xbh"][0:128, :])
        nc.sync.dma_start(W4[0][:], io["w4"][0:128, :])
        nc.sync.dma_start(XB[1][:], io["xbh"][128:256, :])
        nc.sync.dma_start(W4[1][:], io["w4"][128:256, :])
        nc.sync.dma_start(MISC[:], io["misc"][:, :])
        nc.sync.dma_start(OH97[:], io["oh97"][:, :])
        for q in range(4):
            xdma(q)

        # ---------------- projections ----------------
        # QhP[p]: one head PAIR per [64, N] tile (PE matmul operands must
        # start at partition 0/32/64). Q biased by bq/s via the ACT copy.
        # Only the t=0 channel half is projected up front; t=1 (pairs 2,3)
        # and the V projection are interleaved into pairs 0/1 so the
        # attention loop starts as early as possible.
        QhP = [ptile(f"QhP{p}", [64, N], F32R) for p in range(4)]
        KhP = [ptile(f"KhP{p}", [64, N], F32R) for p in range(4)]
        VH = [ptile(f"VH{mc}", [128, NH * 33], BF16) for mc in range(NH)]
        for mc in range(8):
            nc.gpsimd.memset(VH[mc][:], 1.0)  # ones cols survive the copy

        def emit_qk_proj(t):
            for dst, w, bias in ((QhP, WQT, BQ), (KhP, WKT, None)):
                pp = bigps(f"pp_{t}_{0 if bias is not None else 1}")
                for jn in range(2):
                    for kc in range(2):
                        nc.tensor.matmul(
                            pp[:, jn * 512 : (jn + 1) * 512],
                            (w[kc][:, t * 128 : (t + 1) * 128]),
                            (XB[kc][:, jn * 512 : (jn + 1) * 512]),
                            start=(kc == 0),
                            stop=(kc == 1),
                        )
                for half in range(2):
                    p = 2 * t + half
                    hs = slice(64 * half, 64 * half + 64)
                    if bias is not None:
                        nc.scalar.activation(
                            dst[p][:], pp[hs, :], IDENT,
                            bias=bias[hs, t : t + 1],
                        )
                    else:
                        nc.scalar.copy(dst[p][:], pp[hs, :])

        def emit_v_proj(mc):
            # VH[mc][:, 33h:33h+32] = V^T rows mc*128.., head h
            pv = bigps(f"pv_{mc}", [128, C])
            for kc in range(2):
                nc.tensor.matmul(
                    pv[:],
                    (XB[kc][:, mc * 128 : mc * 128 + 128]),
                    (WVT[kc][:]),
                    start=(kc == 0),
                    stop=(kc == 1),
                )
            vh3 = VH[mc].rearrange("p (h c) -> p h c", c=33)
            nc.scalar.copy(
                vh3[:, :, 0:32], pv.rearrange("p (h d) -> p h d", d=32)
            )

        emit_qk_proj(0)
        emit_v_proj(0)
        emit_v_proj(1)

        # ---------------- attention ----------------
        sched = _exp_engine_schedule()
        O1 = [ptile(f"O1{t}", [128, N], BF16) for t in range(2)]
        # jn blocks at partition 0/32 (matmul operand base must be 0/32/64)
        RC = [ptile(f"RC{p}", [34, 512], F32R) for p in range(4)]
        DEN = [ptile(f"DEN{p}", [34, 512]) for p in range(4)]
        OST = [ptile(f"OST{jn}", [97, 512]) for jn in range(2)]  # pair hand-off
        PRS = [ptile(f"PRS{jn}", [97, 512]) for jn in range(2)]
        si = 0
        prev = None  # (p, ost-ready) of previous pair
        for p in range(4):  # head pairs (2p, 2p+1)
            psO = [
                psp.tile([97, 512], F32, tag="psO", bufs=2, name=f"psO_{p}_{jn}")
                for jn in range(2)
            ]
            def emit_av(mc, ets):
                for jn in range(2):
                    for hh in range(2):
                        h = 2 * p + hh
                        nc.tensor.matmul(
                            psO[jn][64 * hh : 64 * hh + 33, :],
                            (VH[mc][:, 33 * h : 33 * h + 33]),
                            (ets[jn][:, hh * 512 : (hh + 1) * 512]),
                            start=(mc == 0),
                            stop=(mc == 7),
                            tile_position=(0, 64 * hh),
                            skip_group_check=True,
                        )

            # depth-2 software pipeline: S(mc)+exp(mc) emitted before
            # AV(mc-2), so exps have a full iteration of slack before the
            # PE needs their result.
            etq = []
            for mc in range(8):
                if mc >= 3:
                    emit_av(mc - 3, etq[mc - 3])
                ets = []
                for jn in range(2):
                    ps = psp.tile(
                        [128, N], F32, tag="big", bufs=3, name=f"ps_{p}_{mc}_{jn}"
                    )
                    for hh in range(2):  # head 2p+hh
                        b0 = 32 * hh
                        nc.tensor.matmul(
                            ps[:, hh * 512 : (hh + 1) * 512],
                            (KhP[p][b0 : b0 + 32, mc * 128 : (mc + 1) * 128]),
                            (QhP[p][b0 : b0 + 32, jn * 512 : (jn + 1) * 512]),
                            start=True,
                            stop=True,
                        )
                    et = etp.tile([128, N], BF16, tag="et", name=f"et_{p}_{mc}_{jn}")
                    _exp_emit(nc, sched[si], et, ps)
                    si += 1
                    ets.append(et)
                etq.append(ets)
                if p == 0 and mc < 6:
                    emit_v_proj(mc + 2)
                if p == 1 and mc == 1:
                    emit_qk_proj(1)
                # previous pair's normalization, staggered
                if prev is not None and mc in (3, 4):
                    _normalize_half(
                        nc, psp, RC, OH97, O1, prev, OST, mc - 3, PRS[mc - 3]
                    )
            emit_av(5, etq[5])
            emit_av(6, etq[6])
            emit_av(7, etq[7])
            # hand psO off to SBUF so the accumulators free early, gather
            # the 4 denominator rows via SBUF-SBUF DMA (Sync engine is idle)
            # and take one batched reciprocal (RECIPROCAL cost is free-size
            # based: [4,512] costs the same as [1,512]).
            for jn in range(2):
                nc.scalar.copy(OST[jn][:], psO[jn][:])
            for jn in range(2):
                for hh in range(2):
                    nc.sync.dma_start(
                        DEN[p][32 * jn + hh : 32 * jn + hh + 1, :],
                        OST[jn][64 * hh + 32 : 64 * hh + 33, :],
                    )
            with nc.allow_low_precision("recip of O(100) softmax sums"):
                nc.vector.reciprocal(RC[p][:], DEN[p][:])
            if p < 3:
                prev = p
            else:
                for jn in range(2):
                    _normalize_half(nc, psp, RC, OH97, O1, 3, OST, jn, PRS[jn])

        # ---------------- output projection + residual ----------------
        OUTF = [ptile(f"OUTF{t}", [128, N]) for t in range(2)]
        for jn in range(2):
            for mo in range(2):
                js = slice(jn * 512, (jn + 1) * 512)
                po = psp.tile(
                    [128, 512], F32, tag="big", bufs=3, name=f"po_{mo}_{jn}"
                )
                for kc in range(2):
                    nc.tensor.matmul(
                        po[:],
                        (WOT[kc][:, mo * 128 : (mo + 1) * 128]),
                        (O1[kc][:, js]),
                        start=(kc == 0),
                        stop=(kc == 1),
                    )
                # out = (po + bo2) + x
                nc.vector.scalar_tensor_tensor(
                    OUTF[mo][:, js], po[:], BO2[:, mo : mo + 1],
                    X[2 * mo + jn][:].bitcast(F32), ADD, ADD,
                )
                if jn == 1:
                    nc.sync.dma_start(
                        io["out"][mo * 128 : (mo + 1) * 128, :], OUTF[mo][:]
                    )


def _normalize_half(nc, psp, RC, OH97, O1, p, OST, jn, PRS):
    """Normalize previous pair p from its SBUF copy (OST), n-half jn.
    pr broadcast lands in PSUM; ACT bounces it to SBUF so the multiplies
    can run on Pool (which cannot read PSUM)."""
    t = p // 2
    rb = 64 * (p % 2)
    js = slice(jn * 512, (jn + 1) * 512)
    pr = psp.tile([97, 512], F32, tag="big", bufs=3, name=f"pr_{p}_{jn}")
    nc.tensor.matmul(
        pr[:], (OH97[32 * jn : 32 * jn + 2, :]),
        (RC[p][32 * jn : 32 * jn + 2, :]), start=True, stop=True
    )
    nc.scalar.copy(PRS[:], pr[:])
    for hh in range(2):
        r = rb + 32 * hh
        nc.gpsimd.tensor_tensor(
            O1[t][r : r + 32, js],
            OST[jn][64 * hh : 64 * hh + 32, :],
            PRS[64 * hh : 64 * hh + 32, :],
            MULT,
        )




def build_nc():
    nc = bacc.Bacc("TRN2", target_bir_lowering=False, debug=False)
    io = {}
    for name, shape, dt_ in [
        ("xb", (C, N), F32R),
        ("xbh", (C, N), BF16),
        ("w4", (C, 4 * C), BF16),
        ("misc", (128, 4), F32),
        ("oh97", (34, 97), F32R),
    ]:
        io[name] = nc.dram_tensor(name, shape, dt_, kind="ExternalInput").ap()
    io["out"] = nc.dram_tensor("out", (C, N), F32, kind="ExternalOutput").ap()
    with tile.TileContext(nc) as tc:
        _emit(tc, io)
    nc.finalize()
    return nc


def host_prep(x, Wq, bq, Wk, bk, Wv, bv, Wo, bo):
    """Build per-core input maps (numpy only)."""
    x = np.ascontiguousarray(np.asarray(x, np.float32))
    Wq, bq = np.asarray(Wq, np.float32), np.asarray(bq, np.float32)
    Wk = np.asarray(Wk, np.float32)
    Wv, bv = np.asarray(Wv, np.float32), np.asarray(bv, np.float32)
    Wo, bo = np.asarray(Wo, np.float32), np.asarray(bo, np.float32)
    s = 1.0 / math.sqrt(HD)

    import ml_dtypes
    w4 = np.concatenate([Wq.T * s, Wk.T, Wv.T, Wo.T], axis=1)
    w4 = np.ascontiguousarray(w4.astype(ml_dtypes.bfloat16))
    misc = np.zeros((128, 4), np.float32)
    misc[:, 0:2] = (bq * s).reshape(2, 128).T
    misc[:, 2:4] = (Wo @ bv + bo).reshape(2, 128).T
    oh97 = np.zeros((34, 97), np.float32)
    for r in (0, 32):
        oh97[r, 0:33] = 1.0
        oh97[r + 1, 64:97] = 1.0

    B = x.shape[0]
    in_maps = []
    for b in range(B):
        xb = np.ascontiguousarray(x[b].reshape(C, N))
        in_maps.append(
            {
                "xb": xb,
                "xbh": np.ascontiguousarray(xb.astype(ml_dtypes.bfloat16)),
                "w4": w4,
                "misc": misc,
                "oh97": oh97,
            }
        )
    return in_maps


def kernel(x, Wq, bq, Wk, bk, Wv, bv, Wo, bo):
    global _NC, LAST_RESULTS
    if _NC is None:
        _NC = build_nc()
    in_maps = host_prep(x, Wq, bq, Wk, bk, Wv, bv, Wo, bo)
    res = run_bass_kernel_spmd(_NC, in_maps, core_ids=list(range(NCORES)))
    LAST_RESULTS = res
    out = np.stack([r["out"] for r in res.results], axis=0)
    return out.reshape(NCORES, C, 32, 32).astype(np.float32)


if __name__ == "__main__":
    rng = np.random.default_rng(0)
    ins = {
        "x": rng.standard_normal((8, C, 32, 32), dtype=np.float32),
        "Wq": rng.standard_normal((C, C), dtype=np.float32) / 16,
        "bq": rng.standard_normal(C).astype(np.float32) * 0.01,
        "Wk": rng.standard_normal((C, C), dtype=np.float32) / 16,
        "bk": rng.standard_normal(C).astype(np.float32) * 0.01,
        "Wv": rng.standard_normal((C, C), dtype=np.float32) / 16,
        "bv": rng.standard_normal(C).astype(np.float32) * 0.01,
        "Wo": rng.standard_normal((C, C), dtype=np.float32) / 16,
        "bo": rng.standard_normal(C).astype(np.float32) * 0.01,
    }
    out = kernel(**ins)
    print("out", out.shape, out.dtype, float(np.abs(out).mean()))
